# revision 1
# baseline (speedup 1.0000x reference)
"""Trainium2 Bass kernel for nn_CausalFlowModel (LSTM flow model).

Model (per batch row b, B=2048 rows total):
  h0 = MLP_enc(x[b])                         # 8 -> 256 -> 256 -> 64, tanh/tanh/linear
  h0_stack = [x[b]; h0]                      # 72
  run LSTM (input 9, hidden 72) over T=512 steps from (h0_stack, c0=0)
  dec_in = (1-d)*h[l-2] + d*h[l-1]           # l = h_lens[b], d = deltas[b, l-1]
  out[b] = MLP_dec(dec_in)                   # 72 -> 288 -> 288 -> 8, tanh/tanh/linear

Mapping: pure data parallel over 8 cores (256 rows/core). On-chip layout is
transposed: state tiles are [feature, batch_col].  Per step one fused matmul
(K = 72 h + 9 x + 1 ones = 82) produces the four gates [72, 256] each in PSUM;
sigmoid over [F|I|O] in one ACT op, tanh(G), then the cell update on DVE.

Rows are sorted by h_lens (ascending) and dealt round-robin to cores.  The
h[l-1] / h[l-2] captures are narrow windowed copies on the GpSimd engine whose
column offset is loaded at runtime from a per-core offset table, so the IR is
input-independent.
"""

import math
import os

import numpy as np

import concourse.bacc as bacc
import concourse.bass as bass
import concourse.mybir as mybir
import concourse.tile as tile
from concourse.bass_utils import run_bass_kernel_spmd

F32 = mybir.dt.float32
I32 = mybir.dt.int32
AF = mybir.ActivationFunctionType
ALU = mybir.AluOpType

# Problem constants
B, T, SD, CD = 2048, 512, 8, 8
H = 72          # control_rnn_size + state_dim
NCORES = 8
BC = B // NCORES  # 256 batch columns per core


class Cfg:
    def __init__(self, T=T, bc=BC, W=16, R=4, mm_dt=mybir.dt.float32r,
                 ncores=NCORES):
        self.T = T          # number of LSTM steps
        self.bc = bc        # batch columns per core
        self.W = W          # capture window width
        self.R = R          # rotating hx buffers
        self.mm_dt = mm_dt  # matmul dtype (float32 or float32r)
        self.ncores = ncores
        self.pad = bc + W   # padded column width of capture-read tiles


# --------------------------------------------------------------------------- #
# host-side preparation
# --------------------------------------------------------------------------- #

def _gate_reorder(w4h: np.ndarray) -> np.ndarray:
    """Reorder [4H, ...] from pytorch gate order (i,f,g,o) to (f,i,o,g)."""
    i, f, g, o = np.split(w4h, 4, axis=0)
    return np.concatenate([f, i, o, g], axis=0)


def host_prep(inputs: dict, cfg: Cfg):
    """Sort/deal rows, build per-core input maps (shared weight arrays)."""
    lens = np.asarray(inputs["h_lens"]).astype(np.int64)
    order = np.argsort(lens, kind="stable")

    # shared (replicated) weight tensors ------------------------------------
    W_ih = np.asarray(inputs["W_ih"], np.float32)   # [288, 9]
    W_hh = np.asarray(inputs["W_hh"], np.float32)   # [288, 72]
    b_g = np.asarray(inputs["b_ih"], np.float32) + np.asarray(inputs["b_hh"], np.float32)
    Wg = np.concatenate([W_hh, W_ih, b_g[:, None]], axis=1)   # [288, 82]
    Wg = _gate_reorder(Wg)                                    # (f,i,o,g)
    wg_all = np.ascontiguousarray(Wg.T)                       # [82, 288] lhsT

    def kchunks(wT, chunk=128):
        # split [K, M] along K into <=128 rows
        return [np.ascontiguousarray(wT[s:s + chunk])
                for s in range(0, wT.shape[0], chunk)]

    we1T = np.ascontiguousarray(np.asarray(inputs["enc_W1"], np.float32).T)  # [8, 256]
    we2T = np.ascontiguousarray(np.asarray(inputs["enc_W2"], np.float32).T)  # [256, 256]
    we3T = np.ascontiguousarray(np.asarray(inputs["enc_W3"], np.float32).T)  # [256, 64]
    wd1T = np.ascontiguousarray(np.asarray(inputs["dec_W1"], np.float32).T)  # [72, 288]
    wd2T = np.ascontiguousarray(np.asarray(inputs["dec_W2"], np.float32).T)  # [288, 288]
    wd3T = np.ascontiguousarray(np.asarray(inputs["dec_W3"], np.float32).T)  # [288, 8]

    def bias_cols(b, p=128):
        # [M] -> [p, ceil(M/p)] column-chunked per-partition bias
        ncol = (len(b) + p - 1) // p
        out = np.zeros((p, ncol), np.float32)
        for c in range(ncol):
            seg = b[c * p:(c + 1) * p]
            out[: len(seg), c] = seg
        return out

    shared = {
        "wg_all": wg_all,
        "we1T": we1T,
        "we2Tk0": kchunks(we2T)[0], "we2Tk1": kchunks(we2T)[1],
        "we3Tk0": kchunks(we3T)[0], "we3Tk1": kchunks(we3T)[1],
        "wd1T": wd1T,
        "wd2Tk0": kchunks(wd2T)[0], "wd2Tk1": kchunks(wd2T)[1],
        "wd2Tk2": kchunks(wd2T)[2],
        "wd3Tk0": kchunks(wd3T)[0], "wd3Tk1": kchunks(wd3T)[1],
        "wd3Tk2": kchunks(wd3T)[2],
        "be1": bias_cols(np.asarray(inputs["enc_b1"], np.float32)),
        "be2": bias_cols(np.asarray(inputs["enc_b2"], np.float32)),
        "be3": bias_cols(np.asarray(inputs["enc_b3"], np.float32), p=64),
        "bd1": bias_cols(np.asarray(inputs["dec_b1"], np.float32)),
        "bd2": bias_cols(np.asarray(inputs["dec_b2"], np.float32)),
        "bd3": bias_cols(np.asarray(inputs["dec_b3"], np.float32), p=8),
    }

    x = np.asarray(inputs["x"], np.float32)
    rnn = np.asarray(inputs["rnn_input"], np.float32)
    deltas = np.asarray(inputs["deltas"], np.float32)

    in_maps, perms = [], []
    maxw = 0
    for k in range(cfg.ncores):
        perm = order[np.arange(cfg.bc) * cfg.ncores + k]
        perms.append(perm)
        lk = lens[perm]
        # capture offset table: tb[u] = #cols with len <= u, u in [0, T+2)
        tb = np.searchsorted(lk, np.arange(cfg.T + 4), side="right").astype(np.int32)
        maxw = max(maxw, int(np.max(np.bincount(lk, minlength=1))))
        dsel = deltas[perm, lk - 1, 0].astype(np.float32)        # [bc]
        d1 = np.broadcast_to(dsel, (H, cfg.bc)).copy()           # weight for h[l-1]
        d2 = np.broadcast_to(1.0 - dsel, (H, cfg.bc)).copy()     # weight for h[l-2]
        rk = rnn[perm].transpose(1, 2, 0)                        # [T, 9, bc]
        rk = np.concatenate(
            [rk, np.ones((cfg.T, 1, cfg.bc), np.float32)], axis=1)   # + ones row
        m = dict(shared)
        m.update({
            "xT": np.ascontiguousarray(x[perm].T),               # [8, bc]
            "rnnT": np.ascontiguousarray(rk).reshape(cfg.T * (CD + 2), cfg.bc),
            "tb": tb.reshape(1, -1),
            "d1": d1,
            "d2": d2,
        })
        in_maps.append(m)
    assert maxw <= cfg.W, f"capture window too small: need {maxw} > {cfg.W}"
    return in_maps, perms


# --------------------------------------------------------------------------- #
# device kernel
# --------------------------------------------------------------------------- #

def build_nc(cfg: Cfg):
    nc = bacc.Bacc("TRN2", target_bir_lowering=False, debug=False,
                   enable_asserts=False, num_devices=cfg.ncores)
    T, bc, W, R, PAD = cfg.T, cfg.bc, cfg.W, cfg.R, cfg.pad

    RD = cfg.mm_dt  # dtype of every matmul operand

    def din(name, shape=None, dt=F32):
        return nc.dram_tensor(name, list(shape), dt, kind="ExternalInput").ap()

    ins = {
        "xT": din("xT", [SD, bc], RD),
        "rnnT": din("rnnT", [T * (CD + 2), bc], RD),
        "tb": din("tb", [1, T + 4], I32),
        "d1": din("d1", [H, bc]),
        "d2": din("d2", [H, bc]),
        "wg_all": din("wg_all", dt=RD, shape=[H + CD + 2, 4 * H]),
        "we1T": din("we1T", dt=RD, shape=[SD, 256]),
        "we2Tk0": din("we2Tk0", dt=RD, shape=[128, 256]), "we2Tk1": din("we2Tk1", dt=RD, shape=[128, 256]),
        "we3Tk0": din("we3Tk0", dt=RD, shape=[128, 64]), "we3Tk1": din("we3Tk1", dt=RD, shape=[128, 64]),
        "wd1T": din("wd1T", dt=RD, shape=[H, 288]),
        "wd2Tk0": din("wd2Tk0", dt=RD, shape=[128, 288]), "wd2Tk1": din("wd2Tk1", dt=RD, shape=[128, 288]),
        "wd2Tk2": din("wd2Tk2", dt=RD, shape=[32, 288]),
        "wd3Tk0": din("wd3Tk0", dt=RD, shape=[128, SD]), "wd3Tk1": din("wd3Tk1", dt=RD, shape=[128, SD]),
        "wd3Tk2": din("wd3Tk2", dt=RD, shape=[32, SD]),
        "be1": din("be1", [128, 2]), "be2": din("be2", [128, 2]),
        "be3": din("be3", [64, 1]),
        "bd1": din("bd1", [128, 3]), "bd2": din("bd2", [128, 3]),
        "bd3": din("bd3", [SD, 1]),
    }
    out_dram = nc.dram_tensor("out", [SD, bc], F32, kind="ExternalOutput").ap()

    KG = H + CD + 2  # 82: [h(72); x(9); ones(1)]

    with tile.TileContext(nc) as tc:
        with tc.tile_pool(name="const", bufs=1) as cpool, \
             tc.tile_pool(name="work", bufs=2) as wpool, \
             tc.tile_pool(name="dram", bufs=2, space="DRAM") as dpool, \
             tc.tile_pool(name="psum", bufs=2, space="PSUM") as ppool:

            # ---- load constants -------------------------------------------
            sb = {}
            for name in ["wg_all", "we1T", "we2Tk0", "we2Tk1", "we3Tk0",
                         "we3Tk1", "wd1T", "wd2Tk0", "wd2Tk1", "wd2Tk2",
                         "wd3Tk0", "wd3Tk1", "wd3Tk2", "be1", "be2", "be3",
                         "bd1", "bd2", "bd3", "d1", "d2", "tb"]:
                ap = ins[name]
                t_ = cpool.tile(list(ap.shape), ap.dtype, name=f"sb_{name}")
                nc.sync.dma_start(t_, ap)
                sb[name] = t_

            # persistent state tiles
            hx = [cpool.tile([KG, PAD], RD, name=f"hx{r}") for r in range(R)]
            CG = cpool.tile([H, 2 * bc], F32, name="CG")   # [c | tanh(g)]
            H1d = nc.dram_tensor("H1d", [H, PAD], F32, kind="ExternalOutput").ap()
            H2d = nc.dram_tensor("H2d", [H, PAD], F32, kind="ExternalOutput").ap()
            hdx = nc.dram_tensor("hdx", [H, PAD], F32, kind="ExternalOutput").ap()
            H1 = cpool.tile([H, PAD], F32, name="H1")      # h[l-1] capture
            H2 = cpool.tile([H, PAD], F32, name="H2")      # h[l-2] capture
            nc.vector.memset(CG[:, 0:bc], 0.0)                      # c0 = 0
            nc.vector.memset(H1, 0.0)
            nc.vector.memset(H2, 0.0)
            for r in range(R):
                nc.vector.memset(hx[r].bitcast(F32), 0.0)

            def mm(out, lhsT, rhs, start=True, stop=True):
                nc.tensor.matmul(out, lhsT, rhs, start=start, stop=stop)

            # ---- encoder MLP: h0 = W3 t(W2 t(W1 x + b1) + b2) + b3 --------
            # x lives in hx[0][0:8, 0:bc]
            nc.sync.dma_start(hx[0][0:SD, 0:bc], ins["xT"])
            ez1p = ppool.tile([128, 512], F32, name="ez1p", tag="ps")
            for c in range(2):
                mm(ez1p[:, 256 * c:256 * (c + 1)],
                   sb["we1T"][:, 128 * c:128 * (c + 1)], hx[0][0:SD, 0:bc])
            ez1 = wpool.tile([128, 512], RD, name="ez1")
            for c in range(2):
                nc.scalar.activation(ez1[:, 256 * c:256 * (c + 1)],
                                     ez1p[:, 256 * c:256 * (c + 1)],
                                     AF.Tanh, bias=sb["be1"][:, c:c + 1])
            ez2p = ppool.tile([128, 512], F32, name="ez2p", tag="ps")
            for c in range(2):
                for k in range(2):
                    mm(ez2p[:, 256 * c:256 * (c + 1)],
                       sb[f"we2Tk{k}"][:, 128 * c:128 * (c + 1)],
                       ez1[:, 256 * k:256 * (k + 1)],
                       start=(k == 0), stop=(k == 1))
            ez2 = wpool.tile([128, 512], RD, name="ez2")
            for c in range(2):
                nc.scalar.activation(ez2[:, 256 * c:256 * (c + 1)],
                                     ez2p[:, 256 * c:256 * (c + 1)],
                                     AF.Tanh, bias=sb["be2"][:, c:c + 1])
            eh0p = ppool.tile([64, 256], F32, name="eh0p", tag="ps")
            for k in range(2):
                mm(eh0p, sb[f"we3Tk{k}"], ez2[:, 256 * k:256 * (k + 1)],
                   start=(k == 0), stop=(k == 1))
            eh0 = wpool.tile([64, 256], RD, name="eh0")
            nc.scalar.activation(eh0, eh0p, AF.Identity, bias=sb["be3"][:, 0:1])
            # shift h0 into hx[0] rows 8:72 (partition shift -> DMA)
            nc.sync.dma_start(hx[0][SD:H, 0:bc], eh0)

            # ---- LSTM over T steps ----------------------------------------
            rnn_ap = ins["rnnT"]
            NX = CD + 2
            for r in range(min(R, T)):
                nc.sync.dma_start(hx[r][H:KG, 0:bc],
                                  rnn_ap[r * NX:(r + 1) * NX, :])

            dbg = os.environ.get("KDBG", "")
            prev_cap = None
            for t in range(T):
                cur = hx[t % R]
                nxt = hx[(t + 1) % R]
                rhs = cur[0:KG, 0:bc]
                gp = ppool.tile([H, 4 * bc], F32, name="gp", tag="ps")
                for c in range(4):  # F, I, O, G
                    mm(gp[:, bc * c:bc * (c + 1)],
                       sb["wg_all"][:, H * c:H * (c + 1)], rhs)
                S = wpool.tile([H, 3 * bc], F32, name="S")
                nc.scalar.activation(S, gp[:, 0:3 * bc], AF.Sigmoid)
                nc.scalar.activation(CG[:, bc:2 * bc], gp[:, 3 * bc:4 * bc],
                                     AF.Tanh)
                Tt = wpool.tile([H, 2 * bc], F32, name="Tt")
                nc.vector.tensor_tensor(Tt, S[:, 0:2 * bc], CG, op=ALU.mult)
                nc.vector.tensor_tensor(CG[:, 0:bc], Tt[:, 0:bc],
                                        Tt[:, bc:2 * bc], op=ALU.add)
                TC = wpool.tile([H, bc], F32, name="TC")
                nc.scalar.activation(TC, CG[:, 0:bc], AF.Tanh)
                nc.vector.tensor_tensor(nxt[0:H, 0:bc], S[:, 2 * bc:3 * bc],
                                        TC, op=ALU.mult)
                # prefetch x for step t+R into the buffer just read
                if t + R < T:
                    nc.sync.dma_start(
                        cur[H:KG, 0:bc],
                        rnn_ap[(t + R) * NX:(t + R + 1) * NX, :])
                # captures: h_t -> DRAM ring; h[l-1] (tb[t]) and h[l-2]
                # (tb[t+1]) as dynamic-offset DRAM->DRAM window copies
                hd = hdx
                nc.sync.dma_start(hd, nxt.bitcast(F32)[0:H, 0:PAD])
                if "nowin" in dbg:
                    prev_cap = None
                    continue
                if "statwin" in dbg:
                    nc.sync.dma_start(H1d[:, 0:W], hd[:, 0:W])
                    nc.sync.dma_start(H2d[:, 0:W], hd[:, 0:W])
                    prev_cap = None
                    continue
                tmp1 = nc.sync.alloc_register(f"cap1_{t}")
                if "movwin" in dbg:
                    ld1 = nc.sync.reg_mov(tmp1, 0)
                else:
                    ld1 = nc.sync.reg_load(tmp1, sb["tb"][0:1, t:t + 1])
                if prev_cap is not None:
                    tile.add_dep_helper(ld1.ins, prev_cap.ins, sync=False,
                                        reason="cap reg chain")
                v1 = nc.sync.snap(tmp1, donate=True)
                tmp2 = nc.sync.alloc_register(f"cap2_{t}")
                if "movwin" in dbg:
                    ld2 = nc.sync.reg_mov(tmp2, 0)
                else:
                    ld2 = nc.sync.reg_load(tmp2, sb["tb"][0:1, t + 1:t + 2])
                v2 = nc.sync.snap(tmp2, donate=True)
                cp1 = nc.sync.dma_start(H1d[:, bass.ds(v1, W)],
                                        hd[:, bass.ds(v1, W)],
                                        cond=None if "nocond" in dbg
                                        else (v2 > v1))
                prev_cap = nc.sync.dma_start(H2d[:, bass.ds(v2, W)],
                                             hd[:, bass.ds(v2, W)])

            nc.sync.dma_start(H1[:, 0:bc], H1d[:, 0:bc])
            nc.sync.dma_start(H2[:, 0:bc], H2d[:, 0:bc])

            # ---- dec_in = d1*h[l-1] + d2*h[l-2] ---------------------------
            U1 = wpool.tile([H, bc], F32, name="U1")
            nc.vector.tensor_tensor(U1, sb["d1"], H1[:, 0:bc], op=ALU.mult)
            U2 = wpool.tile([H, bc], F32, name="U2")
            nc.vector.tensor_tensor(U2, sb["d2"], H2[:, 0:bc], op=ALU.mult)
            DI = wpool.tile([H, bc], RD, name="DI")
            nc.vector.tensor_tensor(DI, U1, U2, op=ALU.add)

            # ---- decoder MLP ----------------------------------------------
            CH1 = [(0, 128), (128, 128), (256, 32)]
            dz1p = ppool.tile([128, 768], F32, name="dz1p", tag="ps")
            for c, (off, m) in enumerate(CH1):
                mm(dz1p[0:m, 256 * c:256 * c + bc], sb["wd1T"][:, off:off + m], DI)
            dz1 = wpool.tile([128, 768], RD, name="dz1")
            for c, (off, m) in enumerate(CH1):
                nc.scalar.activation(dz1[0:m, 256 * c:256 * c + bc],
                                     dz1p[0:m, 256 * c:256 * c + bc],
                                     AF.Tanh, bias=sb["bd1"][0:m, c:c + 1])
            dz2p = ppool.tile([128, 768], F32, name="dz2p", tag="ps")
            for c, (off, m) in enumerate(CH1):
                for k, (koff, km) in enumerate(CH1):
                    mm(dz2p[0:m, 256 * c:256 * c + bc],
                       sb[f"wd2Tk{k}"][0:km, off:off + m],
                       dz1[0:km, 256 * k:256 * k + bc],
                       start=(k == 0), stop=(k == 2))
            dz2 = wpool.tile([128, 768], RD, name="dz2")
            for c, (off, m) in enumerate(CH1):
                nc.scalar.activation(dz2[0:m, 256 * c:256 * c + bc],
                                     dz2p[0:m, 256 * c:256 * c + bc],
                                     AF.Tanh, bias=sb["bd2"][0:m, c:c + 1])
            dz3p = ppool.tile([SD, 256], F32, name="dz3p", tag="ps")
            for k, (koff, km) in enumerate(CH1):
                mm(dz3p, sb[f"wd3Tk{k}"][0:km, :],
                   dz2[0:km, 256 * k:256 * k + bc],
                   start=(k == 0), stop=(k == 2))
            OUT = wpool.tile([SD, bc], F32, name="OUT")
            nc.scalar.activation(OUT, dz3p, AF.Identity, bias=sb["bd3"][:, 0:1])
            nc.sync.dma_start(out_dram, OUT)

    nc.compile()
    return nc, ins, out_dram


# --------------------------------------------------------------------------- #
# entry point
# --------------------------------------------------------------------------- #

def kernel(**inputs) -> np.ndarray:
    cfg = Cfg()
    lens = np.asarray(inputs["h_lens"]).astype(np.int64)
    maxcnt = int(np.max(np.bincount(lens, minlength=1)))
    if maxcnt > cfg.W:
        cfg.W = 1 << int(math.ceil(math.log2(maxcnt)))
        cfg.pad = cfg.bc + cfg.W
    in_maps, perms = host_prep(inputs, cfg)
    nc, _, _ = build_nc(cfg)
    res = run_bass_kernel_spmd(nc, in_maps, core_ids=list(range(cfg.ncores)))
    out = np.empty((B, SD), np.float32)
    for k in range(cfg.ncores):
        out[perms[k]] = res.results[k]["out"].T
    return out



# revision 13
# speedup vs baseline: 2.1615x; 2.1615x over previous
"""Trainium2 Bass kernel for nn_CausalFlowModel (LSTM flow model).

Model (per batch row b, B=2048 rows total):
  h0 = MLP_enc(x[b])                         # 8 -> 256 -> 256 -> 64, tanh/tanh/linear
  h0_stack = [x[b]; h0]                      # 72
  run LSTM (input 9, hidden 72) over T=512 steps from (h0_stack, c0=0)
  dec_in = (1-d)*h[l-2] + d*h[l-1]           # l = h_lens[b], d = deltas[b, l-1]
  out[b] = MLP_dec(dec_in)                   # 72 -> 288 -> 288 -> 8, tanh/tanh/linear

Mapping: pure data parallel over 8 cores (256 rows/core), feature-on-partition
layout (state tiles are [feature, batch_col]).  Per core the 256 columns are
sorted by h_lens and parity-dealt into two groups of 128 that are software-
pipelined against each other so PE/ACT/DVE overlap across groups.

All gates go through a single tanh (sigmoid(z) = (tanh(z/2)+1)/2, the 1/2
pre-scales folded into the weights host-side; the hidden state is kept doubled,
h2 = 2h, and the cell state doubled, S = 2c, so the whole cell update is three
fused scalar_tensor_tensor ops).  Dead columns (those whose h_lens has passed)
are statically sliced away: the compile-time schedule is derived from the
actual h_lens, shared across all 8 cores (union of per-core bounds).

h[l-1]/h[l-2] captures are mask-predicated SBUF copies (copy_predicated) over
narrow static windows; masks are per-core input data, so one IR serves all
cores. No DRAM round trips, no register-patched DMAs.
"""

import ml_dtypes
import numpy as np

import concourse.bacc as bacc
import concourse.bass as bass
import concourse.mybir as mybir
import concourse.tile as tile
from concourse.bass_utils import run_bass_kernel_spmd

F32 = mybir.dt.float32
RD = mybir.dt.float32r
BF = mybir.dt.bfloat16
AF = mybir.ActivationFunctionType
ALU = mybir.AluOpType

# Problem constants
B, T, SD, CD = 2048, 512, 8, 8
H = 72          # control_rnn_size + state_dim
NCORES = 8
BC = B // NCORES   # 256 batch columns per core
G = 2              # pipeline groups per core
GW = BC // G       # 128 columns per group
NX = CD + 2        # rnn input rows per step (9 + ones)
KG = H + NX        # 82 contraction rows: [h2(72); x(9); 1]


class Cfg:
    def __init__(self):
        self.T = T
        self.bc = BC
        self.R = 4          # rotating h2/x ring buffers


# --------------------------------------------------------------------------- #
# host-side preparation
# --------------------------------------------------------------------------- #

def host_prep(inputs: dict, cfg: Cfg):
    """Sort/deal rows, build shared schedule + per-core input maps."""
    lens = np.asarray(inputs["h_lens"]).astype(np.int64)
    order = np.argsort(lens, kind="stable")

    # physical column layout per core: slot s (sorted ascending) -> group s%2,
    # group-index s//2; group g occupies physical cols [g*GW, (g+1)*GW)
    slot_of_phys = np.empty(BC, np.int64)
    slot_of_phys[:GW] = 2 * np.arange(GW)
    slot_of_phys[GW:] = 2 * np.arange(GW) + 1

    perms = []          # per-core: physical col -> original batch row
    lgs = []            # per-core per-group sorted lengths [8][2][GW]
    for k in range(NCORES):
        perm_sorted = order[np.arange(BC) * NCORES + k]    # ascending by len
        perm_phys = perm_sorted[slot_of_phys]
        perms.append(perm_phys)
        lk = lens[perm_sorted]
        lgs.append([lk[0::2], lk[1::2]])

    # ---- shared static schedule (union over cores) ------------------------
    tgrid = np.arange(T + 3)
    sched = {"lo": [], "cap1": [], "cap2": [], "L1": [], "L2": []}
    mask_specs = []  # per group: list of (kind, t, wlo, whi, off)
    for g in range(G):
        tb = np.stack([np.searchsorted(lgs[k][g], tgrid, side="right")
                       for k in range(NCORES)])            # [8, T+3]
        lo = tb.min(axis=0)                                # [T+3]
        sched["lo"].append(lo)
        cap1, cap2 = [], []
        off1 = off2 = 0
        for t in range(T):
            w1lo, w1hi = int(lo[t]), int(tb[:, t + 1].max())
            if w1hi > w1lo:
                cap1.append((t, w1lo, w1hi, off1))
                off1 += w1hi - w1lo
            w2lo, w2hi = int(tb[:, t + 1].min()), int(tb[:, t + 2].max())
            if w2hi > w2lo:
                cap2.append((t, w2lo, w2hi, off2))
                off2 += w2hi - w2lo
        sched["cap1"].append(cap1)
        sched["cap2"].append(cap2)
        sched["L1"].append(off1)
        sched["L2"].append(off2)

    # ---- shared weights ---------------------------------------------------
    W_ih = np.asarray(inputs["W_ih"], np.float64)   # [288, 9] gate order i,f,g,o
    W_hh = np.asarray(inputs["W_hh"], np.float64)   # [288, 72]
    bsum = (np.asarray(inputs["b_ih"], np.float64)
            + np.asarray(inputs["b_hh"], np.float64))
    gi = {"i": 0, "f": 1, "g": 2, "o": 3}
    wg = np.zeros((KG, 4 * H), np.float64)          # col blocks (f, i, g, o)
    for c, gate in enumerate("figo"):
        r = slice(gi[gate] * H, (gi[gate] + 1) * H)
        s1, s2 = (0.5, 1.0) if gate == "g" else (0.25, 0.5)
        wg[0:H, c * H:(c + 1) * H] = s1 * W_hh[r].T
        wg[H:H + CD + 1, c * H:(c + 1) * H] = s2 * W_ih[r].T
        wg[KG - 1, c * H:(c + 1) * H] = s2 * bsum[r]
    wg = wg.astype(np.float32)

    def kchunks(wT, chunk=128):
        return [np.ascontiguousarray(wT[s:s + chunk])
                for s in range(0, wT.shape[0], chunk)]

    we1T = np.ascontiguousarray(np.asarray(inputs["enc_W1"], np.float32).T)
    we2T = np.ascontiguousarray(np.asarray(inputs["enc_W2"], np.float32).T)
    we3T = np.ascontiguousarray(2.0 * np.asarray(inputs["enc_W3"], np.float32).T)
    wd1T = np.ascontiguousarray(np.asarray(inputs["dec_W1"], np.float32).T)
    wd2T = np.ascontiguousarray(np.asarray(inputs["dec_W2"], np.float32).T)
    wd3T = np.ascontiguousarray(np.asarray(inputs["dec_W3"], np.float32).T)

    def bias_cols(b, p=128):
        ncol = (len(b) + p - 1) // p
        out = np.zeros((p, ncol), np.float32)
        for c in range(ncol):
            seg = b[c * p:(c + 1) * p]
            out[: len(seg), c] = seg
        return out

    bf = ml_dtypes.bfloat16
    shared = {
        "wg": wg.astype(bf),
        "we1T": we1T,
        "we2Tk0": kchunks(we2T)[0], "we2Tk1": kchunks(we2T)[1],
        "we3Tk0": kchunks(we3T)[0], "we3Tk1": kchunks(we3T)[1],
        "wd1T": wd1T,
        "wd2Tk0": kchunks(wd2T)[0], "wd2Tk1": kchunks(wd2T)[1],
        "wd2Tk2": kchunks(wd2T)[2],
        "wd3Tk0": kchunks(wd3T)[0], "wd3Tk1": kchunks(wd3T)[1],
        "wd3Tk2": kchunks(wd3T)[2],
        "be1": bias_cols(np.asarray(inputs["enc_b1"], np.float32)),
        "be2": bias_cols(np.asarray(inputs["enc_b2"], np.float32)),
        "be3": bias_cols(2.0 * np.asarray(inputs["enc_b3"], np.float32), p=64),
        "bd1": bias_cols(np.asarray(inputs["dec_b1"], np.float32)),
        "bd2": bias_cols(np.asarray(inputs["dec_b2"], np.float32)),
        "bd3": bias_cols(np.asarray(inputs["dec_b3"], np.float32), p=SD),
    }

    x = np.asarray(inputs["x"], np.float32)
    rnn = np.asarray(inputs["rnn_input"], np.float32)
    deltas = np.asarray(inputs["deltas"], np.float32)

    in_maps = []
    for k in range(NCORES):
        perm = perms[k]
        lk = lens[perm]
        dsel = deltas[perm, lk - 1, 0].astype(np.float32)
        d1 = np.broadcast_to(0.5 * dsel, (H, BC)).copy()          # h[l-1] weight
        d2 = np.broadcast_to(0.5 * (1.0 - dsel), (H, BC)).copy()  # h[l-2] weight
        rk = rnn[perm].transpose(1, 2, 0)                         # [T, 9, BC]
        rk = np.concatenate([rk, np.ones((T, 1, BC), np.float32)], axis=1)
        m = dict(shared)
        m.update({
            "xT": np.ascontiguousarray(x[perm].T),                # [8, BC]
            "rnnT": np.ascontiguousarray(rk).reshape(T * NX, BC).astype(bf),
            "d1": d1.astype(bf), "d2": d2.astype(bf),
        })
        # capture masks
        for g in range(G):
            lg = lgs[k][g]
            m1 = np.zeros((H, max(sched["L1"][g], 1)), np.float32)
            for (t, wlo, whi, off) in sched["cap1"][g]:
                m1[:, off:off + whi - wlo] = (lg[wlo:whi] == t + 1)[None, :]
            m2 = np.zeros((H, max(sched["L2"][g], 1)), np.float32)
            for (t, wlo, whi, off) in sched["cap2"][g]:
                m2[:, off:off + whi - wlo] = (lg[wlo:whi] == t + 2)[None, :]
            m[f"m1g{g}"] = m1.astype(np.uint8)
            m[f"m2g{g}"] = m2.astype(np.uint8)
        in_maps.append(m)
    return in_maps, perms, sched


# --------------------------------------------------------------------------- #
# device kernel
# --------------------------------------------------------------------------- #

def build_nc(cfg: Cfg, sched, dbg=False):
    nc = bacc.Bacc("TRN2", target_bir_lowering=False, debug=False,
                   enable_asserts=False, num_devices=NCORES)
    R = cfg.R

    def din(name, shape, dt=F32):
        return nc.dram_tensor(name, list(shape), dt, kind="ExternalInput").ap()

    ins = {
        "xT": din("xT", [SD, BC], RD),
        "rnnT": din("rnnT", [T * NX, BC], BF),
        "d1": din("d1", [H, BC], BF),
        "d2": din("d2", [H, BC], BF),
        "wg": din("wg", [KG, 4 * H], BF),
        "we1T": din("we1T", [SD, 256], RD),
        "we2Tk0": din("we2Tk0", [128, 256], RD), "we2Tk1": din("we2Tk1", [128, 256], RD),
        "we3Tk0": din("we3Tk0", [128, 64], RD), "we3Tk1": din("we3Tk1", [128, 64], RD),
        "wd1T": din("wd1T", [H, 288], RD),
        "wd2Tk0": din("wd2Tk0", [128, 288], RD), "wd2Tk1": din("wd2Tk1", [128, 288], RD),
        "wd2Tk2": din("wd2Tk2", [32, 288], RD),
        "wd3Tk0": din("wd3Tk0", [128, SD], RD), "wd3Tk1": din("wd3Tk1", [128, SD], RD),
        "wd3Tk2": din("wd3Tk2", [32, SD], RD),
        "be1": din("be1", [128, 2]), "be2": din("be2", [128, 2]),
        "be3": din("be3", [64, 1]),
        "bd1": din("bd1", [128, 3]), "bd2": din("bd2", [128, 3]),
        "bd3": din("bd3", [SD, 1]),
    }
    for g in range(G):
        ins[f"m1g{g}"] = din(f"m1g{g}", [H, max(sched["L1"][g], 1)], mybir.dt.uint8)
        ins[f"m2g{g}"] = din(f"m2g{g}", [H, max(sched["L2"][g], 1)], mybir.dt.uint8)
    out_dram = nc.dram_tensor("out", [SD, BC], F32, kind="ExternalOutput").ap()

    # per-step capture lookup: (t, g) -> (wlo, whi, off)
    cap1 = [dict((t, (a, b, o)) for (t, a, b, o) in sched["cap1"][g])
            for g in range(G)]
    cap2 = [dict((t, (a, b, o)) for (t, a, b, o) in sched["cap2"][g])
            for g in range(G)]
    lo_t = sched["lo"]

    with tile.TileContext(nc) as tc:
        with tc.tile_pool(name="const", bufs=1) as cpool, \
             tc.tile_pool(name="work", bufs=4) as wpool, \
             tc.tile_pool(name="psum", bufs=4, space="PSUM") as ppool:

            # ---- load constants -------------------------------------------
            sb = {}
            for name, ap in ins.items():
                if name == "rnnT":
                    continue            # streamed from DRAM per step
                t_ = cpool.tile(list(ap.shape), ap.dtype, name=f"sb_{name}")
                nc.sync.dma_start(t_, ap)
                sb[name] = t_

            # persistent state
            hx = [cpool.tile([KG, BC], BF, name=f"hx{r}") for r in range(R)]
            U = [cpool.tile([H, 5, GW], BF, name=f"U{g}") for g in range(G)]
            H1 = cpool.tile([H, BC], BF, name="H1")
            H2 = cpool.tile([H, BC], BF, name="H2")
            for g in range(G):
                nc.vector.memset(U[g], 0.0)     # S block = 2*c0 = 0
            nc.vector.memset(H1, 0.0)
            nc.vector.memset(H2, 0.0)

            def mm(out, lhsT, rhs, start=True, stop=True):
                nc.tensor.matmul(out, lhsT, rhs, start=start, stop=stop)

            # ---- encoder MLP: h0 = W3 t(W2 t(W1 x + b1) + b2) + b3 --------
            ez1p = ppool.tile([128, 512], F32, name="ez1p", tag="ps")
            for c in range(2):
                mm(ez1p[:, 256 * c:256 * (c + 1)],
                   sb["we1T"][:, 128 * c:128 * (c + 1)], sb["xT"])
            ez1 = wpool.tile([128, 512], RD, name="ez1")
            for c in range(2):
                nc.scalar.activation(ez1[:, 256 * c:256 * (c + 1)],
                                     ez1p[:, 256 * c:256 * (c + 1)],
                                     AF.Tanh, bias=sb["be1"][:, c:c + 1])
            ez2p = ppool.tile([128, 512], F32, name="ez2p", tag="ps")
            for c in range(2):
                for k in range(2):
                    mm(ez2p[:, 256 * c:256 * (c + 1)],
                       sb[f"we2Tk{k}"][:, 128 * c:128 * (c + 1)],
                       ez1[:, 256 * k:256 * (k + 1)],
                       start=(k == 0), stop=(k == 1))
            ez2 = wpool.tile([128, 512], RD, name="ez2")
            for c in range(2):
                nc.scalar.activation(ez2[:, 256 * c:256 * (c + 1)],
                                     ez2p[:, 256 * c:256 * (c + 1)],
                                     AF.Tanh, bias=sb["be2"][:, c:c + 1])
            eh0p = ppool.tile([64, 256], F32, name="eh0p", tag="ps")
            for k in range(2):
                mm(eh0p, sb[f"we3Tk{k}"], ez2[:, 256 * k:256 * (k + 1)],
                   start=(k == 0), stop=(k == 1))
            eh0 = wpool.tile([64, 256], F32, name="eh0")
            nc.scalar.activation(eh0, eh0p, AF.Identity, bias=sb["be3"][:, 0:1])
            # h2(0) = 2*[x; h0]: rows 0:8 = 2x (ACT copy), rows 8:72 via DMA
            nc.scalar.activation(hx[0][0:SD, :], sb["xT"].bitcast(F32),
                                 AF.Identity, scale=2.0)
            nc.gpsimd.dma_start(hx[0][SD:H, :], eh0)   # cast f32 -> bf16
            for r in range(R):
                nc.sync.dma_start(hx[r][H:KG, :], ins["rnnT"][r * NX:(r + 1) * NX, :])

            # ---- LSTM over T steps ----------------------------------------
            for t in range(T):
                cur = hx[t % R]
                nxt = hx[(t + 1) % R]
                gps, tcs, act_done = [None] * G, [None] * G, [None] * G
                for g in range(G):
                    lo = int(lo_t[g][t])
                    if lo >= GW:
                        gps[g] = None
                        continue
                    gb = g * GW
                    gp = ppool.tile([H, 4, GW], F32, name="gp", tag="ps")
                    rhs = cur[0:KG, gb + lo:gb + GW]
                    for c in range(4):  # f, i, g, o
                        mm(gp[:, c, lo:], sb["wg"][:, H * c:H * (c + 1)], rhs)
                    gps[g] = (gp, lo, gb)
                # one tanh for all four gates (pre-scales folded in weights)
                for g in range(G):
                    if gps[g] is None:
                        continue
                    gp, lo, gb = gps[g]
                    nc.scalar.activation(U[g][:, 1:5, lo:], gp[:, :, lo:], AF.Tanh)
                # AB = ([tf|ti] + 1) * [S|tg]
                Vs = [None] * G
                for g in range(G):
                    if gps[g] is None:
                        continue
                    _, lo, gb = gps[g]
                    V = wpool.tile([H, 2, GW], BF, name="V")
                    nc.vector.scalar_tensor_tensor(
                        V[:, :, lo:], U[g][:, 1:3, lo:], 1.0, U[g][:, 0:4:3, lo:],
                        op0=ALU.add, op1=ALU.mult)
                    Vs[g] = V
                # S' = 0.5*A + B   (= 2c')
                for g in range(G):
                    if gps[g] is None:
                        continue
                    _, lo, gb = gps[g]
                    nc.vector.scalar_tensor_tensor(
                        U[g][:, 0, lo:], Vs[g][:, 0, lo:], 0.5, Vs[g][:, 1, lo:],
                        op0=ALU.mult, op1=ALU.add)
                # tc = tanh(c') = tanh(0.5 * S')
                for g in range(G):
                    if gps[g] is None:
                        continue
                    _, lo, gb = gps[g]
                    TC = wpool.tile([H, GW], BF, name="TC")
                    nc.scalar.activation(TC[:, lo:], U[g][:, 0, lo:], AF.Tanh,
                                         scale=0.5)
                    tcs[g] = TC
                # h2' = (to + 1) * tc   (= 2h)
                for g in range(G):
                    if gps[g] is None:
                        continue
                    _, lo, gb = gps[g]
                    nc.vector.scalar_tensor_tensor(
                        nxt[0:H, gb + lo:gb + GW], U[g][:, 4, lo:], 1.0,
                        tcs[g][:, lo:], op0=ALU.add, op1=ALU.mult)
                # captures: H1 <- h2' where l-1 == t, H2 <- h2' where l-2 == t
                for g in range(G):
                    gb = g * GW
                    c1 = cap1[g].get(t)
                    if c1 is not None:
                        wlo, whi, off = c1
                        nc.vector.copy_predicated(
                            H1[:, gb + wlo:gb + whi],
                            sb[f"m1g{g}"][:, off:off + whi - wlo],
                            nxt[0:H, gb + wlo:gb + whi])
                    c2 = cap2[g].get(t)
                    if c2 is not None:
                        wlo, whi, off = c2
                        nc.vector.copy_predicated(
                            H2[:, gb + wlo:gb + whi],
                            sb[f"m2g{g}"][:, off:off + whi - wlo],
                            nxt[0:H, gb + wlo:gb + whi])
                # prefetch x rows for step t+R into the buffer just read
                if t + R < T:
                    nc.sync.dma_start(
                        cur[H:KG, :],
                        ins["rnnT"][(t + R) * NX:(t + R + 1) * NX, :])
                if dbg and t == 0:
                    dbg_h2 = nc.dram_tensor("dbg_h2", [KG, BC], BF,
                                            kind="ExternalOutput").ap()
                    nc.sync.dma_start(dbg_h2, nxt)
                    dbg_U0 = nc.dram_tensor("dbg_U0", [H, 5, GW], BF,
                                            kind="ExternalOutput").ap()
                    nc.sync.dma_start(dbg_U0, U[0])

            if dbg:
                for nm, t_ in (("dbg_H1", H1), ("dbg_H2", H2)):
                    dap = nc.dram_tensor(nm, [H, BC], BF,
                                         kind="ExternalOutput").ap()
                    nc.sync.dma_start(dap, t_)

            # ---- dec_in = d1*H1 + d2*H2 (weights pre-halved: H* hold 2h) --
            U1 = wpool.tile([H, BC], BF, name="U1")
            nc.vector.tensor_tensor(U1, sb["d1"], H1, op=ALU.mult)
            U2 = wpool.tile([H, BC], BF, name="U2")
            nc.vector.tensor_tensor(U2, sb["d2"], H2, op=ALU.mult)
            DI = wpool.tile([H, BC], RD, name="DI")
            nc.vector.tensor_tensor(DI, U1, U2, op=ALU.add)

            # ---- decoder MLP ----------------------------------------------
            CH1 = [(0, 128), (128, 128), (256, 32)]
            dz1p = ppool.tile([128, 768], F32, name="dz1p", tag="ps")
            for c, (off, m) in enumerate(CH1):
                mm(dz1p[0:m, 256 * c:256 * c + BC], sb["wd1T"][:, off:off + m], DI)
            dz1 = wpool.tile([128, 768], RD, name="dz1")
            for c, (off, m) in enumerate(CH1):
                nc.scalar.activation(dz1[0:m, 256 * c:256 * c + BC],
                                     dz1p[0:m, 256 * c:256 * c + BC],
                                     AF.Tanh, bias=sb["bd1"][0:m, c:c + 1])
            dz2p = ppool.tile([128, 768], F32, name="dz2p", tag="ps")
            for c, (off, m) in enumerate(CH1):
                for k, (koff, km) in enumerate(CH1):
                    mm(dz2p[0:m, 256 * c:256 * c + BC],
                       sb[f"wd2Tk{k}"][0:km, off:off + m],
                       dz1[0:km, 256 * k:256 * k + BC],
                       start=(k == 0), stop=(k == 2))
            dz2 = wpool.tile([128, 768], RD, name="dz2")
            for c, (off, m) in enumerate(CH1):
                nc.scalar.activation(dz2[0:m, 256 * c:256 * c + BC],
                                     dz2p[0:m, 256 * c:256 * c + BC],
                                     AF.Tanh, bias=sb["bd2"][0:m, c:c + 1])
            dz3p = ppool.tile([SD, 256], F32, name="dz3p", tag="ps")
            for k, (koff, km) in enumerate(CH1):
                mm(dz3p, sb[f"wd3Tk{k}"][0:km, :],
                   dz2[0:km, 256 * k:256 * k + BC],
                   start=(k == 0), stop=(k == 2))
            OUT = wpool.tile([SD, BC], F32, name="OUT")
            nc.scalar.activation(OUT, dz3p, AF.Identity, bias=sb["bd3"][:, 0:1])
            nc.sync.dma_start(out_dram, OUT)
            if dbg:
                for nm, t_ in (("dbg_DI", DI), ("dbg_dz1", dz1),
                               ("dbg_dz2", dz2)):
                    dap = nc.dram_tensor(nm, list(t_.shape), BF,
                                         kind="ExternalOutput").ap()
                    nc.sync.dma_start(dap, t_)

    nc.compile()
    return nc, ins, out_dram


# --------------------------------------------------------------------------- #
# entry point
# --------------------------------------------------------------------------- #

def kernel(**inputs) -> np.ndarray:
    cfg = Cfg()
    in_maps, perms, sched = host_prep(inputs, cfg)
    nc, _, _ = build_nc(cfg, sched)
    res = run_bass_kernel_spmd(nc, in_maps, core_ids=list(range(NCORES)))
    out = np.empty((B, SD), np.float32)
    for k in range(NCORES):
        out[perms[k]] = res.results[k]["out"].T
    return out
